# revision 15
# baseline (speedup 1.0000x reference)
"""BrahmanAttention Trainium2 kernel.

Multi-head attention with a per-head case-pair bias (gathered via one-hot
augmentation of the QK contraction) and a per-head verb-position bias
(folded into the exp activation as a per-partition bias).

Sharding: core c = (batch b = c//2, head-half g = c%2). Each of the 8
NeuronCores computes one batch x 8 heads. Wq/Wk/Wv are column-sharded and Wo
row-sharded by head group, so each core emits a partial [L, D] output; the
host sums the two partials per batch and adds the constant row bv @ Wo + bo.

All per-core input tensors are packed into ONE flat fp32 DRAM tensor ("pk")
addressed with access patterns, so one dispatch carries only 2 operands
(pk, out) -- per-operand dispatch overhead through the PJRT tunnel is ~1 ms.

x and the weights are packed as bf16 (halves HBM traffic; inputs are loaded
with ONE DMA per tensor, spread across the SP/ACT/DVE HWDGE rings so the
load phase overlaps). All matmuls run at 1 cycle/row either way (bf16 and
f32r with free>=256 are the same speed on the PE); PSUM accumulation stays
fp32, so the bf16 rounding costs ~0.3% relative error against a 2e-2 gate.

Device layouts (per core):
  xt_all  [128, (e f)] bf16   x[b]^T in 8 e-chunks of 128 D-rows
  Q^T/K^T produced head-wise as qat_h/kat_h [72, L] bf16:
          rows 0:64  = head's Q^T/SCALE (+bq/SCALE via tensor_scalar) / K^T
          rows 64:72 = one-hot(case_ids) (Q side) / case_bias[h] @ onehot^T
  S^T     [L_j, L_i] per head via one 72-contraction matmul per tile
  exp     on ScalarE with bias = verb_bias[h] * verb_mask[b, j], es bf16
  AV      lhsT = [V_h | ones] bf16 so the denominator Z arrives as row 64
  O^T     normalized by 1/Z (K=1 matmul broadcast of the reciprocal row)
  out     partial = O^T.T @ Wo_shard, accumulated over 4x128 d-chunks
"""

import sys

if "/opt/trn_rl_repo" not in sys.path:
    sys.path.insert(0, "/opt/trn_rl_repo")

import numpy as np

B, L, D, H = 4, 1024, 1024, 16
HD = D // H            # 64
NUM_CASES = 8
SCALE = 8.0            # sqrt(HD)
HPC = 8                # heads per core
DHC = HPC * HD         # 512 head-dims per core
NCORES = 8
KAUG = HD + NUM_CASES  # 72 augmented contraction dim

# ---- packed input layout (fp32 element offsets; bf16 regions bitcast) ----
_XT = 0                          # [128, 8*1024] bf16 x[b]^T (p, e, f)
_WQ = _XT + D * L // 2           # [128, 8*512] bf16 Wq[:, cols]/SCALE (p,e,f)
_WK = _WQ + D * DHC // 2         # [128, 8*512] bf16
_WV = _WK + D * DHC // 2         # [128, 8*512] bf16
_WO = _WV + D * DHC // 2         # [128, 4*1024] bf16 Wo[cols, :] (p, dc, f)
_EOH = _WO + DHC * D // 2        # [8, 1024] bf16 one-hot(case_ids)
_CTK = _EOH + NUM_CASES * L // 2  # [8, 64] bf16 case_bias[hs] repacked
_VBH = _CTK + NUM_CASES * HPC * NUM_CASES // 2  # [128, 64] f32 verb bias grid
_BQ = _VBH + 128 * HPC * 8       # [128, 4] f32 bq[cols]/SCALE per dhb chunk
_NPK = _BQ + 128 * 4

_cached = {}


def _build_nc(repeat=1, enable_partition_id=False):
    import concourse.bass as bass
    import concourse.tile as tile
    from concourse import bacc, mybir
    from contextlib import ExitStack

    f32 = mybir.dt.float32
    f32r = mybir.dt.float32r
    bf16 = mybir.dt.bfloat16
    Exp = mybir.ActivationFunctionType.Exp

    nc = bacc.Bacc("TRN2", target_bir_lowering=False, debug=False,
                   num_devices=NCORES,
                   enable_partition_id=enable_partition_id)

    pk_d = nc.dram_tensor("pk", [_NPK], f32r, kind="ExternalInput")
    out_d = nc.dram_tensor("out", [L, D], f32, kind="ExternalOutput")

    def pk2(off, p, f):
        return pk_d[off:off + p * f].rearrange("(p f) -> p f", p=p)

    def pk2h(off, p, f):
        """bf16 region: off in f32 words, p*f bf16 elements."""
        return pk_d[off:off + p * f // 2].bitcast(bf16).rearrange(
            "(p f) -> p f", p=p)

    def mm(out, lhsT, rhs, start, stop):
        nc.tensor.matmul(out, lhsT, rhs, start=start, stop=stop)

    def body(rep):
        with tile.TileContext(nc) as tc, ExitStack() as ctx:
            pp = ctx.enter_context(
                tc.tile_pool(name=f"persist{rep}", bufs=1))

            va = [pp.tile([128, HPC * (HD + 1)], bf16, name=f"va{jb}",
                          tag=f"va{jb}") for jb in range(8)]
            otf = [pp.tile([128, L], bf16, name=f"otf{dc}", tag=f"otf{dc}")
                   for dc in range(4)]
            vb_sb = pp.tile([128, HPC, 8], f32, name="vb", tag="vb")
            eoh_sb = pp.tile([NUM_CASES, L], bf16, name="eoh", tag="eoh")
            ctk_sb = pp.tile([NUM_CASES, HPC * NUM_CASES], bf16, name="ctk",
                             tag="ctk")
            kstage = pp.tile([HPC * NUM_CASES, L], bf16, name="kstage",
                             tag="kstage")
            bq_sb = pp.tile([128, 4], f32, name="bq", tag="bq")
            ones_row = pp.tile([1, HD], f32r, name="ones", tag="ones")

            # tiny kaug inputs first (PE warms up on them), then xt
            nc.sync.dma_start(out=eoh_sb, in_=pk2h(_EOH, NUM_CASES, L))
            nc.sync.dma_start(out=ctk_sb,
                              in_=pk2h(_CTK, NUM_CASES, HPC * NUM_CASES))
            nc.vector.memset(ones_row, 1.0)

            with tc.tile_pool(name=f"wp{rep}", bufs=1) as wp, \
                 tc.tile_pool(name=f"qk{rep}", bufs=2) as qk, \
                 tc.tile_pool(name=f"attn{rep}", bufs=6) as ap_, \
                 tc.tile_pool(name=f"attn1{rep}", bufs=1) as a1, \
                 tc.tile_pool(name=f"ost{rep}", bufs=2) as ost, \
                 tc.tile_pool(name=f"sps{rep}", bufs=3, space="PSUM") as sps, \
                 tc.tile_pool(name=f"otps{rep}", bufs=1, space="PSUM") as otp:

                # ---- bulk loads: one DMA per tensor, one per DGE ring ----
                xt_all = wp.tile([128, 8 * L], bf16, name="xt", tag="xt")
                wq_all = wp.tile([128, 8 * DHC], bf16, name="wq", tag="wq")
                wk_all = wp.tile([128, 8 * DHC], bf16, name="wk", tag="wk")
                wv_all = wp.tile([128, 8 * DHC], bf16, name="wv", tag="wv")
                wo_all = wp.tile([128, 4 * D], bf16, name="wo", tag="wo")
                # xt in 4 chunk-DMAs so the V accumulation starts nibbling
                # as soon as the first e-pair lands
                xt_dram = pk2h(_XT, 128, 8 * L)
                for q in range(4):
                    nc.sync.dma_start(
                        out=xt_all[:, q * 2 * L:(q + 1) * 2 * L],
                        in_=xt_dram[:, q * 2 * L:(q + 1) * 2 * L])
                nc.scalar.dma_start(out=wq_all, in_=pk2h(_WQ, 128, 8 * DHC))
                nc.gpsimd.dma_start(out=wv_all, in_=pk2h(_WV, 128, 8 * DHC))
                nc.gpsimd.dma_start(out=wk_all, in_=pk2h(_WK, 128, 8 * DHC))
                nc.scalar.dma_start(out=wo_all, in_=pk2h(_WO, 128, 4 * D))
                nc.sync.dma_start(out=bq_sb,
                                  in_=pk2(_BQ, 128, 4).bitcast(f32))
                nc.sync.dma_start(
                    out=vb_sb,
                    in_=pk2(_VBH, 128, HPC * 8).bitcast(f32)
                    .rearrange("p (h jb) -> p h jb", jb=8))

                def xt_c(e):     # [128, L] chunk e of x^T
                    return xt_all[:, e * L:(e + 1) * L]

                def w_c(w, e):   # [128, DHC] chunk e of a projection weight
                    return w[:, e * DHC:(e + 1) * DHC]

                def ps512():
                    """[128, 512] PSUM scratch carved from the shared ring."""
                    return sps.tile([128, L], f32, name="s", tag="s")[:, 0:512]

                # kaug = stacked case_bias[h]^T @ onehot^T -> [64 (h,c), L]
                kg_ps = sps.tile([128, L], f32, name="s", tag="s")
                for ih in range(2):
                    sl = slice(ih * 512, ih * 512 + 512)
                    mm(kg_ps[0:HPC * NUM_CASES, sl], ctk_sb, eoh_sb[:, sl],
                       True, True)
                nc.vector.tensor_copy(kstage, kg_ps[0:HPC * NUM_CASES, :])

                # V: [128 j, 512 dh] accumulated over e; evacuate straight
                # into va's stride-65 layout (ones column via memset)
                for jb in range(8):
                    va_r = va[jb].rearrange("p (h c) -> p h c", c=HD + 1)
                    nc.gpsimd.memset(va_r[:, :, HD:HD + 1], 1.0)
                    jsl = slice(jb * 128, jb * 128 + 128)
                    ps = ps512()
                    for e in range(8):
                        mm(ps, xt_c(e)[:, jsl], w_c(wv_all, e), e == 0,
                           e == 7)
                    nc.scalar.copy(
                        va_r[:, :, 0:HD],
                        ps.rearrange("p (h c) -> p h c", c=HD))

                def project(w_all, dhb, dst_pair, bq_dhb):
                    """one 128-dh block (= head pair) of Q^T or K^T"""
                    csl = slice(dhb * 128, dhb * 128 + 128)
                    evacs = []
                    for ih in range(2):
                        isl = slice(ih * 512, ih * 512 + 512)
                        ps = ps512()
                        for e in range(8):
                            mm(ps, w_c(w_all, e)[:, csl], xt_c(e)[:, isl],
                               e == 0, e == 7)
                        evacs.append((ih, ps))
                    for half in range(2):
                        hsl = slice(64 * half, 64 * half + 64)
                        for ih, ps in evacs:
                            isl = slice(ih * 512, ih * 512 + 512)
                            if bq_dhb is not None:
                                nc.vector.tensor_scalar_add(
                                    dst_pair[half][0:HD, isl], ps[hsl, :],
                                    bq_dhb[hsl, :])
                            else:
                                nc.vector.tensor_copy(
                                    dst_pair[half][0:HD, isl], ps[hsl, :])

                def av(h, jb, ot_ps, es):
                    lh = va[jb][:, h * (HD + 1):(h + 1) * (HD + 1)]
                    for ih in range(2):
                        isl = slice(ih * 512, ih * 512 + 512)
                        mm(ot_ps[:, isl], lh, es[:, isl], jb == 0, jb == 7)

                def head(h, qat_h, kat_h, deferred):
                    """attention for one head; returns deferred-tail closure"""
                    ot_ps = otp.tile([HD + 1, L], f32, name="ot", tag="ot")
                    pend = {}
                    for jb in range(8):
                        jsl = slice(jb * 128, jb * 128 + 128)
                        s_ps = sps.tile([128, L], f32, name="s", tag="s")
                        for ih in range(2):
                            isl = slice(ih * 512, ih * 512 + 512)
                            mm(s_ps[:, isl], kat_h[:, jsl], qat_h[:, isl],
                               True, True)
                        es = ap_.tile([128, L], bf16, name="es", tag="es")
                        for ih in range(2):   # ACT cannot read across banks
                            isl = slice(ih * 512, ih * 512 + 512)
                            nc.scalar.activation(es[:, isl], s_ps[:, isl],
                                                 Exp,
                                                 bias=vb_sb[:, h, jb:jb + 1])
                        pend[jb] = es
                        if jb == 2 and deferred is not None:
                            deferred()   # prev head's rzb matmul + normalize
                        if jb >= 4:
                            av(h, jb - 4, ot_ps, pend.pop(jb - 4))
                    for jb in (4, 5, 6, 7):
                        av(h, jb, ot_ps, pend.pop(jb))

                    # evacuate O^T|Z eagerly so the single ot_ps PSUM tile
                    # is free before the next head's first AV; the
                    # normalize runs later off-PSUM.
                    otst = ost.tile([HD + 1, L], f32, name="otst", tag="otst")
                    nc.vector.tensor_copy(otst[:, 0:512], ot_ps[:, 0:512])
                    nc.vector.tensor_copy(otst[:, 512:1024],
                                          ot_ps[:, 512:1024])
                    rz1 = a1.tile([1, L], f32r, name="rz1", tag="rz1")
                    with nc.allow_low_precision(reason="f32r keeps ~17 "
                                                "mantissa bits; 1/Z fine"):
                        for ih in range(2):
                            isl = slice(ih * 512, ih * 512 + 512)
                            nc.vector.reciprocal(rz1[:, isl],
                                                 otst[HD:HD + 1, isl])

                    def tail():
                        # broadcast 1/Z across 64 partitions via K=1 matmul
                        rzb_ps = sps.tile([128, L], f32, name="s",
                                          tag="s")[0:HD, :]
                        for ih in range(2):
                            isl = slice(ih * 512, ih * 512 + 512)
                            mm(rzb_ps[:, isl], ones_row, rz1[:, isl],
                               True, True)
                        nc.vector.tensor_mul(
                            otf[h // 2][64 * (h % 2):64 * (h % 2) + 64, :],
                            otst[0:HD, :], rzb_ps)
                    return tail

                deferred = None
                for dhb in range(4):
                    qpair = [qk.tile([KAUG, L], bf16, name=f"qat{h % 4}",
                                     tag=f"qat{h % 4}")
                             for h in (2 * dhb, 2 * dhb + 1)]
                    kpair = [qk.tile([KAUG, L], bf16, name=f"kat{h % 4}",
                                     tag=f"kat{h % 4}")
                             for h in (2 * dhb, 2 * dhb + 1)]
                    for i, h in enumerate((2 * dhb, 2 * dhb + 1)):
                        nc.sync.dma_start(out=qpair[i][HD:KAUG, :],
                                          in_=eoh_sb[:, :])
                        nc.sync.dma_start(
                            out=kpair[i][HD:KAUG, :],
                            in_=kstage[(h % HPC) * 8:(h % HPC) * 8 + 8, :])
                    project(wq_all, dhb, qpair,
                            bq_sb[:, dhb:dhb + 1])
                    project(wk_all, dhb, kpair, None)
                    for i, h in enumerate((2 * dhb, 2 * dhb + 1)):
                        deferred = head(h, qpair[i], kpair[i], deferred)
                deferred()  # last head's tail

            # ---- output projection --------------------------------------
            # dc=3 (the last head pair's otf) is deferred per 4-tile group
            # so the first 12 matmuls overlap the final head's normalize.
            with tc.tile_pool(name=f"fin{rep}", bufs=3) as fp, \
                 tc.tile_pool(name=f"fps{rep}", bufs=4, space="PSUM") as fps:
                for ibp in range(4):
                    group = []
                    for ib in (2 * ibp, 2 * ibp + 1):
                        isl = slice(ib * 128, ib * 128 + 128)
                        osb = fp.tile([128, D], f32, name="osb", tag="osb")
                        for eh in range(2):
                            esl = slice(eh * 512, eh * 512 + 512)
                            f_ps = fps.tile([128, 512], f32, name="f",
                                            tag="f")
                            for dc in range(3):
                                mm(f_ps, otf[dc][:, isl],
                                   wo_all[:, dc * D + eh * 512:
                                          dc * D + eh * 512 + 512],
                                   dc == 0, False)
                            group.append((ib, isl, eh, esl, f_ps, osb))
                    for ib, isl, eh, esl, f_ps, osb in group:
                        mm(f_ps, otf[3][:, isl],
                           wo_all[:, 3 * D + eh * 512:3 * D + eh * 512 + 512],
                           False, True)
                        nc.vector.tensor_copy(osb[:, esl], f_ps)
                        if eh == 1:
                            nc.sync.dma_start(out=out_d[isl, :], in_=osb)

    for rep in range(repeat):
        body(rep)

    nc.compile()
    return nc


def _get_nc():
    if "nc" not in _cached:
        _cached["nc"] = _build_nc()
    return _cached["nc"]


def make_packed(**inputs):
    """Host-side sharding: per-core packed fp32 arrays + constant row co."""
    x = np.asarray(inputs["x"], np.float32)
    case_ids = np.asarray(inputs["case_ids"])
    verb_mask = np.asarray(inputs["verb_mask"])
    Wq = np.asarray(inputs["Wq"], np.float32)
    bq = np.asarray(inputs["bq"], np.float32)
    Wk = np.asarray(inputs["Wk"], np.float32)
    Wv = np.asarray(inputs["Wv"], np.float32)
    Wo = np.asarray(inputs["Wo"], np.float32)
    bo = np.asarray(inputs["bo"], np.float32)
    bv = np.asarray(inputs["bv"], np.float32)
    case_bias = np.asarray(inputs["case_bias"], np.float32)
    verb_bias = np.asarray(inputs["verb_bias"], np.float32)
    # NOTE: bk is exactly absorbed by softmax shift invariance; bv/bo are
    # added on the host as co = bv @ Wo + bo (attention rows sum to 1).
    co = (bv @ Wo + bo).astype(np.float32)

    def h2f(a_bf16):
        """bf16 array -> raw f32 view (pairs of bf16 packed per f32 word)."""
        import jax.numpy as jnp
        b = np.asarray(jnp.asarray(a_bf16, jnp.bfloat16)).view(np.uint16)
        return b.reshape(-1, 2).view(np.uint32).ravel().view(np.float32)

    pks = np.empty((NCORES, _NPK), np.float32)
    for c in range(NCORES):
        b, g = c // 2, c % 2
        cols = slice(g * DHC, (g + 1) * DHC)
        hs = np.arange(g * HPC, (g + 1) * HPC)
        pk = pks[c]
        # [p, e, f] packings for single-DMA loads
        xt = x[b].T.reshape(8, 128, L).transpose(1, 0, 2)     # [p, e, f]
        pk[_XT:_XT + D * L // 2] = h2f(xt)
        wq = (Wq[:, cols] / SCALE).reshape(8, 128, DHC).transpose(1, 0, 2)
        pk[_WQ:_WQ + D * DHC // 2] = h2f(wq)
        wk = Wk[:, cols].reshape(8, 128, DHC).transpose(1, 0, 2)
        pk[_WK:_WK + D * DHC // 2] = h2f(wk)
        wv = Wv[:, cols].reshape(8, 128, DHC).transpose(1, 0, 2)
        pk[_WV:_WV + D * DHC // 2] = h2f(wv)
        wo = Wo[cols, :].reshape(4, 128, D).transpose(1, 0, 2)
        pk[_WO:_WO + DHC * D // 2] = h2f(wo)
        eoh = (case_ids[b][None, :] ==
               np.arange(NUM_CASES)[:, None]).astype(np.float32)
        pk[_EOH:_EOH + NUM_CASES * L // 2] = h2f(eoh)
        ctk = case_bias[hs].transpose(2, 0, 1).reshape(NUM_CASES,
                                                      HPC * NUM_CASES)
        pk[_CTK:_CTK + NUM_CASES * HPC * NUM_CASES // 2] = h2f(ctk)
        # [128 p, (h jb)]: entry = verb_bias[h] * verb_mask[b, jb*128+p]
        pk[_VBH:_VBH + 128 * HPC * 8] = (
            verb_bias[hs][None, :, None] *
            verb_mask[b].reshape(8, 128).T[:, None, :]).ravel()
        # [128 p, 4 dhb] per-partition bq/SCALE for the projection evac
        pk[_BQ:_BQ + 128 * 4] = (
            (bq[cols] / SCALE).reshape(4, 128).T).ravel()
    return pks, co


def gather(outs, co):
    """outs: [8, L, D] per-core partials -> full [B, L, D]."""
    out = np.empty((B, L, D), np.float32)
    for b in range(B):
        out[b] = outs[2 * b] + outs[2 * b + 1] + co
    return out


def _get_exec():
    """Compile (once) the fast-dispatch SPMD executable for the kernel."""
    if "exec" in _cached:
        return _cached["exec"]
    import jax
    from concourse import bass2jax
    import concourse.mybir as mybir
    from jax.experimental.shard_map import shard_map
    from jax.sharding import Mesh, PartitionSpec

    nc = _get_nc()
    bass2jax.install_neuronx_cc_hook()
    partition_name = (nc.partition_id_tensor.name
                      if nc.partition_id_tensor else None)

    in_names, out_names, out_avals = [], [], []
    for alloc in nc.m.functions[0].allocations:
        if not isinstance(alloc, mybir.MemoryLocationSet):
            continue
        name = alloc.memorylocations[0].name
        if alloc.kind == "ExternalInput":
            if name != partition_name:
                in_names.append(name)
        elif alloc.kind == "ExternalOutput":
            out_names.append(name)
            shape = tuple(alloc.tensor_shape)
            dtype = mybir.dt.np(alloc.dtype)
            out_avals.append(jax.core.ShapedArray(shape, dtype))
    assert in_names == ["pk"] and out_names == ["out"]
    in_names_all = (in_names +
                    ([partition_name] if partition_name else []))

    def _body(pk):
        pid = [bass2jax.partition_id_tensor()] if partition_name else []
        outs = list(bass2jax._bass_exec_p.bind(
            pk, *pid,
            out_avals=tuple(out_avals),
            in_names=tuple(in_names_all),
            out_names=tuple(out_names),
            lowering_input_output_aliases=(),
            sim_require_finite=True,
            sim_require_nnan=True,
            nc=nc,
        ))
        return tuple(outs)

    devices = jax.devices()[:NCORES]
    mesh = Mesh(np.asarray(devices), ("core",))
    sm = shard_map(_body, mesh=mesh, in_specs=(PartitionSpec("core"),),
                   out_specs=(PartitionSpec("core"),), check_rep=False)

    from jax.sharding import NamedSharding
    shard = NamedSharding(mesh, PartitionSpec("core"))
    arg_shapes = [
        jax.ShapeDtypeStruct((NCORES * _NPK,), np.float32, sharding=shard),
    ]
    fn = bass2jax.fast_dispatch_compile(
        lambda: jax.jit(sm, keep_unused=True).lower(*arg_shapes).compile())

    _cached["exec"] = (fn, shard)
    return _cached["exec"]


def kernel(**inputs):
    import jax

    fn, shard = _get_exec()
    pks, co = make_packed(**inputs)
    pk_dev = jax.device_put(pks.reshape(NCORES * _NPK), shard)
    (out,) = fn(pk_dev)
    out_np = np.asarray(out).reshape(NCORES, L, D)
    return gather(out_np, co)


# revision 22
# speedup vs baseline: 1.0735x; 1.0735x over previous
"""BrahmanAttention Trainium2 kernel.

Multi-head attention with a per-head case-pair bias (gathered via one-hot
augmentation of the QK contraction) and a per-head verb-position bias
(folded into the exp activation as a per-partition bias).

Sharding: core c = (batch b = c//2, head-half g = c%2). Each of the 8
NeuronCores computes one batch x 8 heads. Wq/Wk/Wv are column-sharded and Wo
row-sharded by head group, so each core emits a partial [L, D] output; the
host sums the two partials per batch and adds the constant row bv @ Wo + bo.

All per-core input tensors are packed into ONE flat fp32 DRAM tensor ("pk")
addressed with access patterns, so one dispatch carries only 2 operands
(pk, out) -- per-operand dispatch overhead through the PJRT tunnel is ~1 ms.

x and the weights are packed as bf16 (halves HBM traffic; inputs are loaded
with ONE DMA per tensor, spread across the SP/ACT/DVE HWDGE rings so the
load phase overlaps). All matmuls run at 1 cycle/row either way (bf16 and
f32r with free>=256 are the same speed on the PE); PSUM accumulation stays
fp32, so the bf16 rounding costs ~0.3% relative error against a 2e-2 gate.

Device layouts (per core):
  xt_all  [128, (e f)] bf16   x[b]^T in 8 e-chunks of 128 D-rows
  Q^T/K^T produced head-wise as qat_h/kat_h [72, L] bf16:
          rows 0:64  = head's Q^T/SCALE (+bq/SCALE via tensor_scalar) / K^T
          rows 64:72 = one-hot(case_ids) (Q side) / case_bias[h] @ onehot^T
  S^T     [L_j, L_i] per head via one 72-contraction matmul per tile
  exp     on ScalarE with bias = verb_bias[h] * verb_mask[b, j], es bf16
  AV      lhsT = [V_h | ones] bf16 so the denominator Z arrives as row 64
  O^T     normalized by 1/Z (K=1 matmul broadcast of the reciprocal row)
  out     partial = O^T.T @ Wo_shard, accumulated over 4x128 d-chunks
"""

import sys

if "/opt/trn_rl_repo" not in sys.path:
    sys.path.insert(0, "/opt/trn_rl_repo")

import numpy as np

B, L, D, H = 4, 1024, 1024, 16
HD = D // H            # 64
NUM_CASES = 8
SCALE = 8.0            # sqrt(HD)
HPC = 8                # heads per core
DHC = HPC * HD         # 512 head-dims per core
NCORES = 8
KAUG = HD + NUM_CASES  # 72 augmented contraction dim

# ---- packed input layout (fp32 element offsets; bf16 regions bitcast) ----
_XT = 0                          # [128, 8*1024] bf16 x[b]^T (p, e, f)
_WQ = _XT + D * L // 2           # [128, 8*512] bf16 Wq[:, cols]/SCALE (p,e,f)
_WK = _WQ + D * DHC // 2         # [128, 8*512] bf16
_WV = _WK + D * DHC // 2         # [128, 8*512] bf16
_WO = _WV + D * DHC // 2         # [128, 4*1024] bf16 Wo[cols, :] (p, dc, f)
_EOH = _WO + DHC * D // 2        # [8, 1024] bf16 one-hot(case_ids)
_CTK = _EOH + NUM_CASES * L // 2  # [8, 64] bf16 case_bias[hs] repacked
_VBH = _CTK + NUM_CASES * HPC * NUM_CASES // 2  # [128, 64] f32 verb bias grid
_BQ = _VBH + 128 * HPC * 8       # [128, 4] f32 bq[cols]/SCALE per dhb chunk
_NPK = _BQ + 128 * 4

_cached = {}


def _build_nc(repeat=1, enable_partition_id=False):
    import concourse.bass as bass
    import concourse.tile as tile
    from concourse import bacc, mybir
    from contextlib import ExitStack

    f32 = mybir.dt.float32
    f32r = mybir.dt.float32r
    bf16 = mybir.dt.bfloat16
    Exp = mybir.ActivationFunctionType.Exp

    nc = bacc.Bacc("TRN2", target_bir_lowering=False, debug=False,
                   num_devices=NCORES,
                   enable_partition_id=enable_partition_id)

    pk_d = nc.dram_tensor("pk", [_NPK], f32r, kind="ExternalInput")
    out_d = nc.dram_tensor("out", [L, D], f32, kind="ExternalOutput")

    def pk2(off, p, f):
        return pk_d[off:off + p * f].rearrange("(p f) -> p f", p=p)

    def pk2h(off, p, f):
        """bf16 region: off in f32 words, p*f bf16 elements."""
        return pk_d[off:off + p * f // 2].bitcast(bf16).rearrange(
            "(p f) -> p f", p=p)

    def mm(out, lhsT, rhs, start, stop):
        nc.tensor.matmul(out, lhsT, rhs, start=start, stop=stop)

    def body(rep):
        with tile.TileContext(nc) as tc, ExitStack() as ctx:
            pp = ctx.enter_context(
                tc.tile_pool(name=f"persist{rep}", bufs=1))

            va = [pp.tile([128, HPC * (HD + 1)], bf16, name=f"va{jb}",
                          tag=f"va{jb}") for jb in range(8)]
            otf = [pp.tile([128, L], bf16, name=f"otf{dc}", tag=f"otf{dc}")
                   for dc in range(4)]
            vb_sb = pp.tile([128, HPC, 8], f32, name="vb", tag="vb")
            eoh_sb = pp.tile([NUM_CASES, L], bf16, name="eoh", tag="eoh")
            ctk_sb = pp.tile([NUM_CASES, HPC * NUM_CASES], bf16, name="ctk",
                             tag="ctk")
            kstage = pp.tile([HPC * NUM_CASES, L], bf16, name="kstage",
                             tag="kstage")
            bq_sb = pp.tile([128, 4], f32, name="bq", tag="bq")
            ones_row = pp.tile([1, HD], f32r, name="ones", tag="ones")

            # tiny kaug inputs first (PE warms up on them), then xt
            nc.sync.dma_start(out=eoh_sb, in_=pk2h(_EOH, NUM_CASES, L))
            nc.sync.dma_start(out=ctk_sb,
                              in_=pk2h(_CTK, NUM_CASES, HPC * NUM_CASES))
            nc.vector.memset(ones_row, 1.0)

            with tc.tile_pool(name=f"wp{rep}", bufs=1) as wp, \
                 tc.tile_pool(name=f"qk{rep}", bufs=2) as qk, \
                 tc.tile_pool(name=f"attn{rep}", bufs=6) as ap_, \
                 tc.tile_pool(name=f"attn1{rep}", bufs=1) as a1, \
                 tc.tile_pool(name=f"ost{rep}", bufs=2) as ost, \
                 tc.tile_pool(name=f"fin{rep}", bufs=1) as fp, \
                 tc.tile_pool(name=f"pps{rep}", bufs=2, space="PSUM") as pps, \
                 tc.tile_pool(name=f"sps{rep}", bufs=2, space="PSUM") as sps, \
                 tc.tile_pool(name=f"otps{rep}", bufs=1, space="PSUM") as otp:

                # ---- bulk loads: one DMA per tensor, one per DGE ring ----
                xt_all = wp.tile([128, 8 * L], bf16, name="xt", tag="xt")
                wq_all = wp.tile([128, 8 * DHC], bf16, name="wq", tag="wq")
                wk_all = wp.tile([128, 8 * DHC], bf16, name="wk", tag="wk")
                wv_all = wp.tile([128, 8 * DHC], bf16, name="wv", tag="wv")
                wo_all = wp.tile([128, 4 * D], bf16, name="wo", tag="wo")
                # xt in 4 chunk-DMAs so the V accumulation starts nibbling
                # as soon as the first e-pair lands
                xt_dram = pk2h(_XT, 128, 8 * L)
                for q in range(4):
                    nc.sync.dma_start(
                        out=xt_all[:, q * 2 * L:(q + 1) * 2 * L],
                        in_=xt_dram[:, q * 2 * L:(q + 1) * 2 * L])
                nc.scalar.dma_start(out=wq_all, in_=pk2h(_WQ, 128, 8 * DHC))
                nc.gpsimd.dma_start(out=wv_all, in_=pk2h(_WV, 128, 8 * DHC))
                nc.gpsimd.dma_start(out=wk_all, in_=pk2h(_WK, 128, 8 * DHC))
                nc.scalar.dma_start(out=wo_all, in_=pk2h(_WO, 128, 4 * D))
                nc.sync.dma_start(out=bq_sb,
                                  in_=pk2(_BQ, 128, 4).bitcast(f32))
                nc.sync.dma_start(
                    out=vb_sb,
                    in_=pk2(_VBH, 128, HPC * 8).bitcast(f32)
                    .rearrange("p (h jb) -> p h jb", jb=8))

                def xt_c(e):     # [128, L] chunk e of x^T
                    return xt_all[:, e * L:(e + 1) * L]

                def w_c(w, e):   # [128, DHC] chunk e of a projection weight
                    return w[:, e * DHC:(e + 1) * DHC]

                # kaug = stacked case_bias[h]^T @ onehot^T -> [64 (h,c), L]
                kg_ps = sps.tile([128, L], f32, name="s", tag="s")
                for ih in range(2):
                    sl = slice(ih * 512, ih * 512 + 512)
                    mm(kg_ps[0:HPC * NUM_CASES, sl], ctk_sb, eoh_sb[:, sl],
                       True, True)
                nc.vector.tensor_copy(kstage, kg_ps[0:HPC * NUM_CASES, :])

                # ones columns of va (disjoint from the V evacuation)
                for jb in range(8):
                    va_r = va[jb].rearrange("p (h c) -> p h c", c=HD + 1)
                    nc.gpsimd.memset(va_r[:, :, HD:HD + 1], 1.0)

                # ---- PE fill queue: bulk matmul work (V, next-dhb
                # projections, output-projection partials) is interleaved
                # into the per-head jb loops so the PE:ACT rate stays
                # balanced and the exp pipeline never backs up.
                fill = []

                def drain(n):
                    for _ in range(n):
                        if fill:
                            fill.pop(0)()

                def v_item(jb):
                    def emit():
                        va_r = va[jb].rearrange("p (h c) -> p h c", c=HD + 1)
                        jsl = slice(jb * 128, jb * 128 + 128)
                        ps = pps.tile([128, 512], f32, name="pp", tag="pp")
                        for e in range(8):
                            mm(ps, xt_c(e)[:, jsl], w_c(wv_all, e), e == 0,
                               e == 7)
                        nc.scalar.copy(
                            va_r[:, :, 0:HD],
                            ps.rearrange("p (h c) -> p h c", c=HD))
                    return emit

                def proj_items(w_all, dhb, dst_pair, bq_dhb):
                    """one 128-dh block (= head pair) of Q^T or K^T, as two
                    fill items of 4 matmuls each per ih"""
                    csl = slice(dhb * 128, dhb * 128 + 128)
                    items = []
                    for ih in range(2):
                        isl = slice(ih * 512, ih * 512 + 512)
                        box = {}

                        def first(ih=ih, isl=isl, box=box):
                            ps = pps.tile([128, 512], f32, name="pp",
                                          tag="pp")
                            box["ps"] = ps
                            for e in range(4):
                                mm(ps, w_c(w_all, e)[:, csl],
                                   xt_c(e)[:, isl], e == 0, False)

                        def second(ih=ih, isl=isl, box=box):
                            ps = box["ps"]
                            for e in range(4, 8):
                                mm(ps, w_c(w_all, e)[:, csl],
                                   xt_c(e)[:, isl], False, e == 7)
                            for half in range(2):
                                hsl = slice(64 * half, 64 * half + 64)
                                if bq_dhb is not None:
                                    nc.vector.tensor_scalar_add(
                                        dst_pair[half][0:HD, isl],
                                        ps[hsl, :], bq_dhb[hsl, :])
                                else:
                                    nc.vector.tensor_copy(
                                        dst_pair[half][0:HD, isl],
                                        ps[hsl, :])
                        items += [first, second]
                    return items

                def av(h, jb, ot_ps, es):
                    lh = va[jb][:, h * (HD + 1):(h + 1) * (HD + 1)]
                    for ih in range(2):
                        isl = slice(ih * 512, ih * 512 + 512)
                        mm(ot_ps[:, isl], lh, es[:, isl], jb == 0, jb == 7)

                def head(h, qat_h, kat_h, deferred, post_deferred=None):
                    """attention for one head; returns deferred-tail closure"""
                    ot_ps = otp.tile([HD + 1, L], f32, name="ot", tag="ot")
                    pend = {}
                    for jb in range(8):
                        jsl = slice(jb * 128, jb * 128 + 128)
                        s_ps = sps.tile([128, L], f32, name="s", tag="s")
                        for ih in range(2):
                            isl = slice(ih * 512, ih * 512 + 512)
                            mm(s_ps[:, isl], kat_h[:, jsl], qat_h[:, isl],
                               True, True)
                        es = ap_.tile([128, L], bf16, name="es", tag="es")
                        for ih in range(2):   # ACT cannot read across banks
                            isl = slice(ih * 512, ih * 512 + 512)
                            nc.scalar.activation(es[:, isl], s_ps[:, isl],
                                                 Exp,
                                                 bias=vb_sb[:, h, jb:jb + 1])
                        pend[jb] = es
                        if jb == 2:
                            if deferred is not None:
                                deferred()  # prev head's normalize
                            if post_deferred is not None:
                                post_deferred()
                        drain(1)
                        if jb >= 4:
                            av(h, jb - 4, ot_ps, pend.pop(jb - 4))
                    for jb in (4, 5, 6, 7):
                        av(h, jb, ot_ps, pend.pop(jb))

                    # evacuate O^T|Z eagerly so the single ot_ps PSUM tile
                    # is free before the next head's first AV; the
                    # normalize runs later off-PSUM.
                    otst = ost.tile([HD + 1, L], f32, name="otst", tag="otst")
                    nc.vector.tensor_copy(otst[:, 0:512], ot_ps[:, 0:512])
                    nc.vector.tensor_copy(otst[:, 512:1024],
                                          ot_ps[:, 512:1024])
                    rz1 = a1.tile([1, L], f32r, name="rz1", tag="rz1")
                    with nc.allow_low_precision(reason="f32r keeps ~17 "
                                                "mantissa bits; 1/Z fine"):
                        for ih in range(2):
                            isl = slice(ih * 512, ih * 512 + 512)
                            nc.vector.reciprocal(rz1[:, isl],
                                                 otst[HD:HD + 1, isl])

                    def tail():
                        # broadcast 1/Z across 64 partitions via K=1 matmul
                        rzb_ps = sps.tile([128, L], f32, name="s",
                                          tag="s")[0:HD, :]
                        for ih in range(2):
                            isl = slice(ih * 512, ih * 512 + 512)
                            mm(rzb_ps[:, isl], ones_row, rz1[:, isl],
                               True, True)
                        nc.vector.tensor_mul(
                            otf[h // 2][64 * (h % 2):64 * (h % 2) + 64, :],
                            otst[0:HD, :], rzb_ps)
                    return tail

                def queue_proj(dhb):
                    qpair = [qk.tile([KAUG, L], bf16, name=f"qat{h % 4}",
                                     tag=f"qat{h % 4}")
                             for h in (2 * dhb, 2 * dhb + 1)]
                    kpair = [qk.tile([KAUG, L], bf16, name=f"kat{h % 4}",
                                     tag=f"kat{h % 4}")
                             for h in (2 * dhb, 2 * dhb + 1)]
                    for i, h in enumerate((2 * dhb, 2 * dhb + 1)):
                        nc.sync.dma_start(out=qpair[i][HD:KAUG, :],
                                          in_=eoh_sb[:, :])
                        nc.sync.dma_start(
                            out=kpair[i][HD:KAUG, :],
                            in_=kstage[(h % HPC) * 8:(h % HPC) * 8 + 8, :])
                    fill.extend(proj_items(wq_all, dhb, qpair,
                                           bq_sb[:, dhb:dhb + 1]))
                    fill.extend(proj_items(wk_all, dhb, kpair, None))
                    return qpair, kpair

                # output-projection partials (dc=0..2); dc=3 lands after the
                # last head's normalize, merged by a tensor add at the tail
                osb_t = [fp.tile([128, D], f32, name=f"osb{ib}",
                                 tag=f"osb{ib}") for ib in range(8)]

                def fin_item(ib, eh):
                    def emit():
                        isl = slice(ib * 128, ib * 128 + 128)
                        esl = slice(eh * 512, eh * 512 + 512)
                        f_ps = pps.tile([128, 512], f32, name="pp", tag="pp")
                        for dc in range(3):
                            mm(f_ps, otf[dc][:, isl],
                               wo_all[:, dc * D + eh * 512:
                                      dc * D + eh * 512 + 512],
                               dc == 0, dc == 2)
                        nc.vector.tensor_copy(osb_t[ib][:, esl], f_ps)
                    return emit

                def queue_fin():
                    for ib in range(8):
                        for eh in range(2):
                            fill.append(fin_item(ib, eh))

                # prologue: projections for dhb0 run inline; V and the later
                # projections stream through the fill queue (proj for pair
                # k+1 is queued before pair k's heads and drained there)
                qp, kp = queue_proj(0)
                drain(len(fill))
                fill.extend(v_item(jb) for jb in range(8))
                pend_pair = queue_proj(1)

                deferred = None
                for dhb in range(4):
                    for i, h in enumerate((2 * dhb, 2 * dhb + 1)):
                        post = queue_fin if (dhb == 3 and i == 0) else None
                        deferred = head(h, qp[i], kp[i], deferred, post)
                    drain(len(fill))  # next pair's proj must be complete
                    if dhb < 3:
                        qp, kp = pend_pair
                        if dhb < 2:
                            pend_pair = queue_proj(dhb + 2)
                deferred()  # last head's tail

                # tail: dc=3 into PSUM, tensor-add into osb on DVE, store
                # per ib alternating the SP and ACT DMA rings
                for ib in range(8):
                    isl = slice(ib * 128, ib * 128 + 128)
                    s2 = sps.tile([128, L], f32, name="s", tag="s")
                    for eh in range(2):
                        esl = slice(eh * 512, eh * 512 + 512)
                        mm(s2[:, esl], otf[3][:, isl],
                           wo_all[:, 3 * D + eh * 512:3 * D + eh * 512 + 512],
                           True, True)
                    nc.vector.tensor_add(osb_t[ib], osb_t[ib], s2)
                    eng = nc.sync if ib % 2 == 0 else nc.scalar
                    eng.dma_start(out=out_d[isl, :], in_=osb_t[ib])

    for rep in range(repeat):
        body(rep)

    nc.compile()
    return nc


def _get_nc():
    if "nc" not in _cached:
        _cached["nc"] = _build_nc()
    return _cached["nc"]


def make_packed(**inputs):
    """Host-side sharding: per-core packed fp32 arrays + constant row co."""
    x = np.asarray(inputs["x"], np.float32)
    case_ids = np.asarray(inputs["case_ids"])
    verb_mask = np.asarray(inputs["verb_mask"])
    Wq = np.asarray(inputs["Wq"], np.float32)
    bq = np.asarray(inputs["bq"], np.float32)
    Wk = np.asarray(inputs["Wk"], np.float32)
    Wv = np.asarray(inputs["Wv"], np.float32)
    Wo = np.asarray(inputs["Wo"], np.float32)
    bo = np.asarray(inputs["bo"], np.float32)
    bv = np.asarray(inputs["bv"], np.float32)
    case_bias = np.asarray(inputs["case_bias"], np.float32)
    verb_bias = np.asarray(inputs["verb_bias"], np.float32)
    # NOTE: bk is exactly absorbed by softmax shift invariance; bv/bo are
    # added on the host as co = bv @ Wo + bo (attention rows sum to 1).
    co = (bv @ Wo + bo).astype(np.float32)

    def h2f(a_bf16):
        """bf16 array -> raw f32 view (pairs of bf16 packed per f32 word)."""
        import jax.numpy as jnp
        b = np.asarray(jnp.asarray(a_bf16, jnp.bfloat16)).view(np.uint16)
        return b.reshape(-1, 2).view(np.uint32).ravel().view(np.float32)

    pks = np.empty((NCORES, _NPK), np.float32)
    for c in range(NCORES):
        b, g = c // 2, c % 2
        cols = slice(g * DHC, (g + 1) * DHC)
        hs = np.arange(g * HPC, (g + 1) * HPC)
        pk = pks[c]
        # [p, e, f] packings for single-DMA loads
        xt = x[b].T.reshape(8, 128, L).transpose(1, 0, 2)     # [p, e, f]
        pk[_XT:_XT + D * L // 2] = h2f(xt)
        wq = (Wq[:, cols] / SCALE).reshape(8, 128, DHC).transpose(1, 0, 2)
        pk[_WQ:_WQ + D * DHC // 2] = h2f(wq)
        wk = Wk[:, cols].reshape(8, 128, DHC).transpose(1, 0, 2)
        pk[_WK:_WK + D * DHC // 2] = h2f(wk)
        wv = Wv[:, cols].reshape(8, 128, DHC).transpose(1, 0, 2)
        pk[_WV:_WV + D * DHC // 2] = h2f(wv)
        wo = Wo[cols, :].reshape(4, 128, D).transpose(1, 0, 2)
        pk[_WO:_WO + DHC * D // 2] = h2f(wo)
        eoh = (case_ids[b][None, :] ==
               np.arange(NUM_CASES)[:, None]).astype(np.float32)
        pk[_EOH:_EOH + NUM_CASES * L // 2] = h2f(eoh)
        ctk = case_bias[hs].transpose(2, 0, 1).reshape(NUM_CASES,
                                                      HPC * NUM_CASES)
        pk[_CTK:_CTK + NUM_CASES * HPC * NUM_CASES // 2] = h2f(ctk)
        # [128 p, (h jb)]: entry = verb_bias[h] * verb_mask[b, jb*128+p]
        pk[_VBH:_VBH + 128 * HPC * 8] = (
            verb_bias[hs][None, :, None] *
            verb_mask[b].reshape(8, 128).T[:, None, :]).ravel()
        # [128 p, 4 dhb] per-partition bq/SCALE for the projection evac
        pk[_BQ:_BQ + 128 * 4] = (
            (bq[cols] / SCALE).reshape(4, 128).T).ravel()
    return pks, co


def gather(outs, co):
    """outs: [8, L, D] per-core partials -> full [B, L, D]."""
    out = np.empty((B, L, D), np.float32)
    for b in range(B):
        out[b] = outs[2 * b] + outs[2 * b + 1] + co
    return out


def _get_exec():
    """Compile (once) the fast-dispatch SPMD executable for the kernel."""
    if "exec" in _cached:
        return _cached["exec"]
    import jax
    from concourse import bass2jax
    import concourse.mybir as mybir
    from jax.experimental.shard_map import shard_map
    from jax.sharding import Mesh, PartitionSpec

    nc = _get_nc()
    bass2jax.install_neuronx_cc_hook()
    partition_name = (nc.partition_id_tensor.name
                      if nc.partition_id_tensor else None)

    in_names, out_names, out_avals = [], [], []
    for alloc in nc.m.functions[0].allocations:
        if not isinstance(alloc, mybir.MemoryLocationSet):
            continue
        name = alloc.memorylocations[0].name
        if alloc.kind == "ExternalInput":
            if name != partition_name:
                in_names.append(name)
        elif alloc.kind == "ExternalOutput":
            out_names.append(name)
            shape = tuple(alloc.tensor_shape)
            dtype = mybir.dt.np(alloc.dtype)
            out_avals.append(jax.core.ShapedArray(shape, dtype))
    assert in_names == ["pk"] and out_names == ["out"]
    in_names_all = (in_names +
                    ([partition_name] if partition_name else []))

    def _body(pk):
        pid = [bass2jax.partition_id_tensor()] if partition_name else []
        outs = list(bass2jax._bass_exec_p.bind(
            pk, *pid,
            out_avals=tuple(out_avals),
            in_names=tuple(in_names_all),
            out_names=tuple(out_names),
            lowering_input_output_aliases=(),
            sim_require_finite=True,
            sim_require_nnan=True,
            nc=nc,
        ))
        return tuple(outs)

    devices = jax.devices()[:NCORES]
    mesh = Mesh(np.asarray(devices), ("core",))
    sm = shard_map(_body, mesh=mesh, in_specs=(PartitionSpec("core"),),
                   out_specs=(PartitionSpec("core"),), check_rep=False)

    from jax.sharding import NamedSharding
    shard = NamedSharding(mesh, PartitionSpec("core"))
    arg_shapes = [
        jax.ShapeDtypeStruct((NCORES * _NPK,), np.float32, sharding=shard),
    ]
    fn = bass2jax.fast_dispatch_compile(
        lambda: jax.jit(sm, keep_unused=True).lower(*arg_shapes).compile())

    _cached["exec"] = (fn, shard)
    return _cached["exec"]


def kernel(**inputs):
    import jax

    fn, shard = _get_exec()
    pks, co = make_packed(**inputs)
    pk_dev = jax.device_put(pks.reshape(NCORES * _NPK), shard)
    (out,) = fn(pk_dev)
    out_np = np.asarray(out).reshape(NCORES, L, D)
    return gather(out_np, co)


# revision 28
# speedup vs baseline: 1.1167x; 1.0403x over previous
"""BrahmanAttention Trainium2 kernel.

Multi-head attention with a per-head case-pair bias (gathered via one-hot
augmentation of the QK contraction) and a per-head verb-position bias
(folded into the exp activation as a per-partition bias).

Sharding: core c = (batch b = c//2, head-half g = c%2). Each of the 8
NeuronCores computes one batch x 8 heads. Wq/Wk/Wv are column-sharded and Wo
row-sharded by head group, so each core emits a partial [L, D] output; the
host sums the two partials per batch and adds the constant row bv @ Wo + bo.

All per-core input tensors are packed into ONE flat fp32 DRAM tensor ("pk")
addressed with access patterns, so one dispatch carries only 2 operands
(pk, out) -- per-operand dispatch overhead through the PJRT tunnel is ~1 ms.

x and the weights are packed as bf16 (halves HBM traffic; inputs are loaded
with ONE DMA per tensor, spread across the SP/ACT/DVE HWDGE rings so the
load phase overlaps). All matmuls run at 1 cycle/row either way (bf16 and
f32r with free>=256 are the same speed on the PE); PSUM accumulation stays
fp32, so the bf16 rounding costs ~0.3% relative error against a 2e-2 gate.

Device layouts (per core):
  xt_all  [128, (e f)] bf16   x[b]^T in 8 e-chunks of 128 D-rows
  Q^T/K^T produced head-wise as qat_h/kat_h [72, L] bf16:
          rows 0:64  = head's Q^T/SCALE (+bq/SCALE via tensor_scalar) / K^T
          rows 64:72 = one-hot(case_ids) (Q side) / case_bias[h] @ onehot^T
  S^T     [L_j, L_i] per head via one 72-contraction matmul per tile
  exp     on ScalarE with bias = verb_bias[h] * verb_mask[b, j], es bf16
  AV      lhsT = [V_h | ones] bf16 so the denominator Z arrives as row 64
  O^T     normalized by 1/Z (K=1 matmul broadcast of the reciprocal row)
  out     partial = O^T.T @ Wo_shard, accumulated over 4x128 d-chunks
"""

import sys

if "/opt/trn_rl_repo" not in sys.path:
    sys.path.insert(0, "/opt/trn_rl_repo")

import numpy as np

B, L, D, H = 4, 1024, 1024, 16
HD = D // H            # 64
NUM_CASES = 8
SCALE = 8.0            # sqrt(HD)
HPC = 8                # heads per core
DHC = HPC * HD         # 512 head-dims per core
NCORES = 8
KAUG = HD + NUM_CASES  # 72 augmented contraction dim

# ---- packed input layout (fp32 element offsets; bf16 regions bitcast) ----
_XT = 0                          # [128, 8*1024] bf16 x[b]^T (p, e, f)
_WQ = _XT + D * L // 2           # [128, 8*512] bf16 Wq[:, cols]/SCALE (p,e,f)
_WK = _WQ + D * DHC // 2         # [128, 8*512] bf16
_WV = _WK + D * DHC // 2         # [128, 8*512] bf16
_WO = _WV + D * DHC // 2         # [128, 4*1024] bf16 Wo[cols, :] (p, dc, f)
_EOH = _WO + DHC * D // 2        # [8, 1024] bf16 one-hot(case_ids)
_CTK = _EOH + NUM_CASES * L // 2  # [8, 64] bf16 case_bias[hs] repacked
_VBH = _CTK + NUM_CASES * HPC * NUM_CASES // 2  # [128, 64] f32 verb bias grid
_BQ = _VBH + 128 * HPC * 8       # [128, 4] f32 bq[cols]/SCALE per dhb chunk
_NPK = _BQ + 128 * 4

_cached = {}


def _build_nc(repeat=1, enable_partition_id=False):
    import concourse.bass as bass
    import concourse.tile as tile
    from concourse import bacc, mybir
    from contextlib import ExitStack

    f32 = mybir.dt.float32
    f32r = mybir.dt.float32r
    bf16 = mybir.dt.bfloat16
    Exp = mybir.ActivationFunctionType.Exp

    nc = bacc.Bacc("TRN2", target_bir_lowering=False, debug=False,
                   num_devices=NCORES,
                   enable_partition_id=enable_partition_id)

    pk_d = nc.dram_tensor("pk", [_NPK], f32r, kind="ExternalInput")
    out_d = nc.dram_tensor("out", [L, D], f32, kind="ExternalOutput")

    def pk2(off, p, f):
        return pk_d[off:off + p * f].rearrange("(p f) -> p f", p=p)

    def pk2h(off, p, f):
        """bf16 region: off in f32 words, p*f bf16 elements."""
        return pk_d[off:off + p * f // 2].bitcast(bf16).rearrange(
            "(p f) -> p f", p=p)

    def mm(out, lhsT, rhs, start, stop):
        nc.tensor.matmul(out, lhsT, rhs, start=start, stop=stop)

    def body(rep):
        with tile.TileContext(nc) as tc, ExitStack() as ctx:
            pp = ctx.enter_context(
                tc.tile_pool(name=f"persist{rep}", bufs=1))

            va = [pp.tile([128, HPC * (HD + 1)], bf16, name=f"va{jb}",
                          tag=f"va{jb}") for jb in range(8)]
            otf = [pp.tile([128, L], bf16, name=f"otf{dc}", tag=f"otf{dc}")
                   for dc in range(4)]
            vb_sb = pp.tile([128, HPC, 8], f32, name="vb", tag="vb")
            eoh_sb = pp.tile([NUM_CASES, L], bf16, name="eoh", tag="eoh")
            ctk_sb = pp.tile([NUM_CASES, HPC * NUM_CASES], bf16, name="ctk",
                             tag="ctk")
            kstage = pp.tile([HPC * NUM_CASES, L], bf16, name="kstage",
                             tag="kstage")
            bq_sb = pp.tile([128, 4], f32, name="bq", tag="bq")
            ones_row = pp.tile([1, HD], f32r, name="ones", tag="ones")

            # tiny kaug inputs first (PE warms up on them), then xt
            nc.sync.dma_start(out=eoh_sb, in_=pk2h(_EOH, NUM_CASES, L))
            nc.sync.dma_start(out=ctk_sb,
                              in_=pk2h(_CTK, NUM_CASES, HPC * NUM_CASES))
            nc.vector.memset(ones_row, 1.0)

            with tc.tile_pool(name=f"wp{rep}", bufs=1) as wp, \
                 tc.tile_pool(name=f"qk{rep}", bufs=2) as qk, \
                 tc.tile_pool(name=f"attn{rep}", bufs=6) as ap_, \
                 tc.tile_pool(name=f"attn1{rep}", bufs=1) as a1, \
                 tc.tile_pool(name=f"ost{rep}", bufs=2) as ost, \
                 tc.tile_pool(name=f"fin{rep}", bufs=1) as fp, \
                 tc.tile_pool(name=f"pps{rep}", bufs=2, space="PSUM") as pps, \
                 tc.tile_pool(name=f"sps{rep}", bufs=2, space="PSUM") as sps, \
                 tc.tile_pool(name=f"otps{rep}", bufs=1, space="PSUM") as otp:

                # ---- bulk loads: one DMA per tensor, one per DGE ring ----
                xt_all = wp.tile([128, 8 * L], bf16, name="xt", tag="xt")
                wq_all = wp.tile([128, 8 * DHC], bf16, name="wq", tag="wq")
                wk_all = wp.tile([128, 8 * DHC], bf16, name="wk", tag="wk")
                wv_all = wp.tile([128, 8 * DHC], bf16, name="wv", tag="wv")
                wo_all = wp.tile([128, 4 * D], bf16, name="wo", tag="wo")
                # xt in 4 chunk-DMAs so the V accumulation starts nibbling
                # as soon as the first e-pair lands
                xt_dram = pk2h(_XT, 128, 8 * L)
                for q in range(4):
                    nc.sync.dma_start(
                        out=xt_all[:, q * 2 * L:(q + 1) * 2 * L],
                        in_=xt_dram[:, q * 2 * L:(q + 1) * 2 * L])
                nc.scalar.dma_start(out=wq_all, in_=pk2h(_WQ, 128, 8 * DHC))
                nc.gpsimd.dma_start(out=wv_all, in_=pk2h(_WV, 128, 8 * DHC))
                nc.gpsimd.dma_start(out=wk_all, in_=pk2h(_WK, 128, 8 * DHC))
                nc.scalar.dma_start(out=wo_all, in_=pk2h(_WO, 128, 4 * D))
                nc.sync.dma_start(out=bq_sb,
                                  in_=pk2(_BQ, 128, 4).bitcast(f32))
                nc.sync.dma_start(
                    out=vb_sb,
                    in_=pk2(_VBH, 128, HPC * 8).bitcast(f32)
                    .rearrange("p (h jb) -> p h jb", jb=8))

                def xt_c(e):     # [128, L] chunk e of x^T
                    return xt_all[:, e * L:(e + 1) * L]

                def w_c(w, e):   # [128, DHC] chunk e of a projection weight
                    return w[:, e * DHC:(e + 1) * DHC]

                # kaug = stacked case_bias[h]^T @ onehot^T -> [64 (h,c), L]
                kg_ps = sps.tile([128, L], f32, name="s", tag="s")
                for ih in range(2):
                    sl = slice(ih * 512, ih * 512 + 512)
                    mm(kg_ps[0:HPC * NUM_CASES, sl], ctk_sb, eoh_sb[:, sl],
                       True, True)
                nc.vector.tensor_copy(kstage, kg_ps[0:HPC * NUM_CASES, :])

                # ones columns of va (disjoint from the V evacuation)
                for jb in range(8):
                    va_r = va[jb].rearrange("p (h c) -> p h c", c=HD + 1)
                    nc.gpsimd.memset(va_r[:, :, HD:HD + 1], 1.0)

                # ---- PE fill queue: bulk matmul work (V, next-dhb
                # projections, output-projection partials) is interleaved
                # into the per-head jb loops so the PE:ACT rate stays
                # balanced and the exp pipeline never backs up.
                fill = []

                def drain(n):
                    for _ in range(n):
                        if fill:
                            fill.pop(0)()

                def v_item(jb):
                    def emit():
                        va_r = va[jb].rearrange("p (h c) -> p h c", c=HD + 1)
                        jsl = slice(jb * 128, jb * 128 + 128)
                        ps = pps.tile([128, 512], f32, name="pp", tag="pp")
                        for e in range(8):
                            mm(ps, xt_c(e)[:, jsl], w_c(wv_all, e), e == 0,
                               e == 7)
                        nc.scalar.copy(
                            va_r[:, :, 0:HD],
                            ps.rearrange("p (h c) -> p h c", c=HD))
                    return emit

                def proj_items(w_all, dhb, dst_pair, bq_dhb):
                    """one 128-dh block (= head pair) of Q^T or K^T, as two
                    fill items of 4 matmuls each per ih"""
                    csl = slice(dhb * 128, dhb * 128 + 128)
                    items = []
                    for ih in range(2):
                        isl = slice(ih * 512, ih * 512 + 512)
                        box = {}

                        def first(ih=ih, isl=isl, box=box):
                            ps = pps.tile([128, 512], f32, name="pp",
                                          tag="pp")
                            box["ps"] = ps
                            for e in range(4):
                                mm(ps, w_c(w_all, e)[:, csl],
                                   xt_c(e)[:, isl], e == 0, False)

                        def second(ih=ih, isl=isl, box=box):
                            ps = box["ps"]
                            for e in range(4, 8):
                                mm(ps, w_c(w_all, e)[:, csl],
                                   xt_c(e)[:, isl], False, e == 7)
                            for half in range(2):
                                hsl = slice(64 * half, 64 * half + 64)
                                if bq_dhb is not None:
                                    nc.vector.tensor_scalar_add(
                                        dst_pair[half][0:HD, isl],
                                        ps[hsl, :], bq_dhb[hsl, :])
                                else:
                                    nc.vector.tensor_copy(
                                        dst_pair[half][0:HD, isl],
                                        ps[hsl, :])
                        items += [first, second]
                    return items

                def av(h, jb, ot_ps, es):
                    lh = va[jb][:, h * (HD + 1):(h + 1) * (HD + 1)]
                    for ih in range(2):
                        isl = slice(ih * 512, ih * 512 + 512)
                        mm(ot_ps[:, isl], lh, es[:, isl], jb == 0, jb == 7)

                def head(h, qat_h, kat_h, deferred, post_deferred=None,
                         drains=(1, 1, 1, 1, 1, 1, 1, 1)):
                    """attention for one head; returns deferred-tail closure"""
                    ot_ps = otp.tile([HD + 1, L], f32, name="ot", tag="ot")
                    pend = {}
                    for jb in range(8):
                        jsl = slice(jb * 128, jb * 128 + 128)
                        s_ps = sps.tile([128, L], f32, name="s", tag="s")
                        for ih in range(2):
                            isl = slice(ih * 512, ih * 512 + 512)
                            mm(s_ps[:, isl], kat_h[:, jsl], qat_h[:, isl],
                               True, True)
                        es = ap_.tile([128, L], bf16, name="es", tag="es")
                        for ih in range(2):   # ACT cannot read across banks
                            isl = slice(ih * 512, ih * 512 + 512)
                            nc.scalar.activation(es[:, isl], s_ps[:, isl],
                                                 Exp,
                                                 bias=vb_sb[:, h, jb:jb + 1])
                        pend[jb] = es
                        if jb == 2:
                            if deferred is not None:
                                deferred()  # prev head's normalize
                            if post_deferred is not None:
                                post_deferred()
                        drain(drains[jb])
                        if jb >= 4:
                            av(h, jb - 4, ot_ps, pend.pop(jb - 4))
                    for jb in (4, 5, 6, 7):
                        av(h, jb, ot_ps, pend.pop(jb))

                    # evacuate O^T|Z eagerly so the single ot_ps PSUM tile
                    # is free before the next head's first AV; the
                    # normalize runs later off-PSUM.
                    otst = ost.tile([HD + 1, L], f32, name="otst", tag="otst")
                    nc.vector.tensor_copy(otst[:, 0:512], ot_ps[:, 0:512])
                    nc.vector.tensor_copy(otst[:, 512:1024],
                                          ot_ps[:, 512:1024])
                    rz1 = a1.tile([1, L], f32r, name="rz1", tag="rz1")
                    with nc.allow_low_precision(reason="f32r keeps ~17 "
                                                "mantissa bits; 1/Z fine"):
                        for ih in range(2):
                            isl = slice(ih * 512, ih * 512 + 512)
                            nc.vector.reciprocal(rz1[:, isl],
                                                 otst[HD:HD + 1, isl])

                    def tail():
                        # broadcast 1/Z across 64 partitions via K=1 matmul
                        rzb_ps = sps.tile([128, L], f32, name="s",
                                          tag="s")[0:HD, :]
                        for ih in range(2):
                            isl = slice(ih * 512, ih * 512 + 512)
                            mm(rzb_ps[:, isl], ones_row, rz1[:, isl],
                               True, True)
                        nc.vector.tensor_mul(
                            otf[h // 2][64 * (h % 2):64 * (h % 2) + 64, :],
                            otst[0:HD, :], rzb_ps)
                    return tail

                def queue_proj(dhb):
                    qpair = [qk.tile([KAUG, L], bf16, name=f"qat{h % 4}",
                                     tag=f"qat{h % 4}")
                             for h in (2 * dhb, 2 * dhb + 1)]
                    kpair = [qk.tile([KAUG, L], bf16, name=f"kat{h % 4}",
                                     tag=f"kat{h % 4}")
                             for h in (2 * dhb, 2 * dhb + 1)]
                    for i, h in enumerate((2 * dhb, 2 * dhb + 1)):
                        nc.sync.dma_start(out=qpair[i][HD:KAUG, :],
                                          in_=eoh_sb[:, :])
                        nc.sync.dma_start(
                            out=kpair[i][HD:KAUG, :],
                            in_=kstage[(h % HPC) * 8:(h % HPC) * 8 + 8, :])
                    fill.extend(proj_items(wq_all, dhb, qpair,
                                           bq_sb[:, dhb:dhb + 1]))
                    fill.extend(proj_items(wk_all, dhb, kpair, None))
                    return qpair, kpair

                # output-projection partials (dc=0..2); dc=3 lands after the
                # last head's normalize, merged by a tensor add at the tail
                osb_t = [fp.tile([128, D], f32, name=f"osb{ib}",
                                 tag=f"osb{ib}") for ib in range(8)]

                def fin_item(ib, eh):
                    def emit():
                        isl = slice(ib * 128, ib * 128 + 128)
                        esl = slice(eh * 512, eh * 512 + 512)
                        f_ps = pps.tile([128, 512], f32, name="pp", tag="pp")
                        for dc in range(3):
                            mm(f_ps, otf[dc][:, isl],
                               wo_all[:, dc * D + eh * 512:
                                      dc * D + eh * 512 + 512],
                               dc == 0, dc == 2)
                        nc.vector.tensor_copy(osb_t[ib][:, esl], f_ps)
                        if ib % 2 == 1 and eh == 1:
                            # odd ibs: store the dc0-2 partial now; dc3 is
                            # patched in by an accumulate-DMA at the tail
                            nc.sync.dma_start(out=out_d[isl, :],
                                              in_=osb_t[ib])
                    return emit

                def queue_fin():
                    for ib in range(8):
                        for eh in range(2):
                            fill.append(fin_item(ib, eh))

                # prologue: V0-1 cover the weight-DMA latency, then dhb0's
                # projections run inline, then V2-3 cover the DVE
                # evacuation of qat0/kat0; the rest streams through fill.
                vit = [v_item(jb) for jb in range(8)]
                vit[0]()
                vit[1]()
                qp, kp = queue_proj(0)
                drain(len(fill))
                vit[2]()
                vit[3]()
                fill.extend(vit[4:])
                pend_pair = queue_proj(1)

                even = (1, 0, 1, 0, 1, 0, 1, 0)
                fin0 = (0, 0, 0, 1, 1, 1, 1, 1)
                fin1 = (2, 2, 2, 1, 1, 1, 1, 2)
                deferred = None
                for dhb in range(4):
                    for i, h in enumerate((2 * dhb, 2 * dhb + 1)):
                        post = queue_fin if (dhb == 3 and i == 0) else None
                        dr = (1, 1, 1, 1, 1, 1, 1, 1)
                        if dhb in (1, 2):
                            dr = even
                        elif dhb == 3:
                            dr = fin0 if i == 0 else fin1
                        deferred = head(h, qp[i], kp[i], deferred, post, dr)
                    drain(len(fill))  # next pair's proj must be complete
                    if dhb < 3:
                        qp, kp = pend_pair
                        if dhb < 2:
                            pend_pair = queue_proj(dhb + 2)
                deferred()  # last head's tail

                # tail: dc=3 into PSUM; even ibs merge on DVE and store on
                # the SP ring, odd ibs (partial already in DRAM) evacuate
                # via ACT and patch DRAM with an accumulate-DMA (SWDGE)
                for ib in range(8):
                    isl = slice(ib * 128, ib * 128 + 128)
                    s2 = sps.tile([128, L], f32, name="s", tag="s")
                    for eh in range(2):
                        esl = slice(eh * 512, eh * 512 + 512)
                        mm(s2[:, esl], otf[3][:, isl],
                           wo_all[:, 3 * D + eh * 512:3 * D + eh * 512 + 512],
                           True, True)
                    if ib % 2 == 0:
                        nc.vector.tensor_add(osb_t[ib], osb_t[ib], s2)
                        nc.sync.dma_start(out=out_d[isl, :], in_=osb_t[ib])
                    else:
                        stg = fp.tile([128, D], f32, name="stg",
                                      tag=f"stg{(ib // 2) % 2}")
                        nc.scalar.copy(stg, s2)
                        nc.gpsimd.dma_start(out=out_d[isl, :], in_=stg,
                                            accum_op=mybir.AluOpType.add)

    for rep in range(repeat):
        body(rep)

    nc.compile()
    return nc


def _get_nc():
    if "nc" not in _cached:
        _cached["nc"] = _build_nc()
    return _cached["nc"]


def make_packed(**inputs):
    """Host-side sharding: per-core packed fp32 arrays + constant row co."""
    x = np.asarray(inputs["x"], np.float32)
    case_ids = np.asarray(inputs["case_ids"])
    verb_mask = np.asarray(inputs["verb_mask"])
    Wq = np.asarray(inputs["Wq"], np.float32)
    bq = np.asarray(inputs["bq"], np.float32)
    Wk = np.asarray(inputs["Wk"], np.float32)
    Wv = np.asarray(inputs["Wv"], np.float32)
    Wo = np.asarray(inputs["Wo"], np.float32)
    bo = np.asarray(inputs["bo"], np.float32)
    bv = np.asarray(inputs["bv"], np.float32)
    case_bias = np.asarray(inputs["case_bias"], np.float32)
    verb_bias = np.asarray(inputs["verb_bias"], np.float32)
    # NOTE: bk is exactly absorbed by softmax shift invariance; bv/bo are
    # added on the host as co = bv @ Wo + bo (attention rows sum to 1).
    co = (bv @ Wo + bo).astype(np.float32)

    def h2f(a_bf16):
        """bf16 array -> raw f32 view (pairs of bf16 packed per f32 word)."""
        import jax.numpy as jnp
        b = np.asarray(jnp.asarray(a_bf16, jnp.bfloat16)).view(np.uint16)
        return b.reshape(-1, 2).view(np.uint32).ravel().view(np.float32)

    pks = np.empty((NCORES, _NPK), np.float32)
    for c in range(NCORES):
        b, g = c // 2, c % 2
        cols = slice(g * DHC, (g + 1) * DHC)
        hs = np.arange(g * HPC, (g + 1) * HPC)
        pk = pks[c]
        # [p, e, f] packings for single-DMA loads
        xt = x[b].T.reshape(8, 128, L).transpose(1, 0, 2)     # [p, e, f]
        pk[_XT:_XT + D * L // 2] = h2f(xt)
        wq = (Wq[:, cols] / SCALE).reshape(8, 128, DHC).transpose(1, 0, 2)
        pk[_WQ:_WQ + D * DHC // 2] = h2f(wq)
        wk = Wk[:, cols].reshape(8, 128, DHC).transpose(1, 0, 2)
        pk[_WK:_WK + D * DHC // 2] = h2f(wk)
        wv = Wv[:, cols].reshape(8, 128, DHC).transpose(1, 0, 2)
        pk[_WV:_WV + D * DHC // 2] = h2f(wv)
        wo = Wo[cols, :].reshape(4, 128, D).transpose(1, 0, 2)
        pk[_WO:_WO + DHC * D // 2] = h2f(wo)
        eoh = (case_ids[b][None, :] ==
               np.arange(NUM_CASES)[:, None]).astype(np.float32)
        pk[_EOH:_EOH + NUM_CASES * L // 2] = h2f(eoh)
        ctk = case_bias[hs].transpose(2, 0, 1).reshape(NUM_CASES,
                                                      HPC * NUM_CASES)
        pk[_CTK:_CTK + NUM_CASES * HPC * NUM_CASES // 2] = h2f(ctk)
        # [128 p, (h jb)]: entry = verb_bias[h] * verb_mask[b, jb*128+p]
        pk[_VBH:_VBH + 128 * HPC * 8] = (
            verb_bias[hs][None, :, None] *
            verb_mask[b].reshape(8, 128).T[:, None, :]).ravel()
        # [128 p, 4 dhb] per-partition bq/SCALE for the projection evac
        pk[_BQ:_BQ + 128 * 4] = (
            (bq[cols] / SCALE).reshape(4, 128).T).ravel()
    return pks, co


def gather(outs, co):
    """outs: [8, L, D] per-core partials -> full [B, L, D]."""
    out = np.empty((B, L, D), np.float32)
    for b in range(B):
        out[b] = outs[2 * b] + outs[2 * b + 1] + co
    return out


def _get_exec():
    """Compile (once) the fast-dispatch SPMD executable for the kernel."""
    if "exec" in _cached:
        return _cached["exec"]
    import jax
    from concourse import bass2jax
    import concourse.mybir as mybir
    from jax.experimental.shard_map import shard_map
    from jax.sharding import Mesh, PartitionSpec

    nc = _get_nc()
    bass2jax.install_neuronx_cc_hook()
    partition_name = (nc.partition_id_tensor.name
                      if nc.partition_id_tensor else None)

    in_names, out_names, out_avals = [], [], []
    for alloc in nc.m.functions[0].allocations:
        if not isinstance(alloc, mybir.MemoryLocationSet):
            continue
        name = alloc.memorylocations[0].name
        if alloc.kind == "ExternalInput":
            if name != partition_name:
                in_names.append(name)
        elif alloc.kind == "ExternalOutput":
            out_names.append(name)
            shape = tuple(alloc.tensor_shape)
            dtype = mybir.dt.np(alloc.dtype)
            out_avals.append(jax.core.ShapedArray(shape, dtype))
    assert in_names == ["pk"] and out_names == ["out"]
    in_names_all = (in_names +
                    ([partition_name] if partition_name else []))

    def _body(pk):
        pid = [bass2jax.partition_id_tensor()] if partition_name else []
        outs = list(bass2jax._bass_exec_p.bind(
            pk, *pid,
            out_avals=tuple(out_avals),
            in_names=tuple(in_names_all),
            out_names=tuple(out_names),
            lowering_input_output_aliases=(),
            sim_require_finite=True,
            sim_require_nnan=True,
            nc=nc,
        ))
        return tuple(outs)

    devices = jax.devices()[:NCORES]
    mesh = Mesh(np.asarray(devices), ("core",))
    sm = shard_map(_body, mesh=mesh, in_specs=(PartitionSpec("core"),),
                   out_specs=(PartitionSpec("core"),), check_rep=False)

    from jax.sharding import NamedSharding
    shard = NamedSharding(mesh, PartitionSpec("core"))
    arg_shapes = [
        jax.ShapeDtypeStruct((NCORES * _NPK,), np.float32, sharding=shard),
    ]
    fn = bass2jax.fast_dispatch_compile(
        lambda: jax.jit(sm, keep_unused=True).lower(*arg_shapes).compile())

    _cached["exec"] = (fn, shard)
    return _cached["exec"]


def kernel(**inputs):
    import jax

    fn, shard = _get_exec()
    pks, co = make_packed(**inputs)
    pk_dev = jax.device_put(pks.reshape(NCORES * _NPK), shard)
    (out,) = fn(pk_dev)
    out_np = np.asarray(out).reshape(NCORES, L, D)
    return gather(out_np, co)
